# revision 88
# baseline (speedup 1.0000x reference)
"""Self-contained Trainium2 Bass kernel for nn_Attention (8-head self-attention).

Reference computation (per batch element b):
    xt = x[b].reshape(C, N).T            # (N, C),  N = H*W = 1024
    q  = xt @ Wq                         # (N, 512)
    k, v = split(xt @ Wkv)               # (N, 512) each
    per head h (d=64): sim = q_h k_h^T / 8 ; P = softmax(sim) ; o_h = P v_h
    out[b] = concat_h(o_h) @ Wo + bo     # (N, C)

Sharding: pure data parallel -- core b computes batch element b (8 cores, 8
batch elements, no collectives).

Layout strategy (keeps every matmul contraction dim on partitions, zero
on-chip transposes):
  - x[b] is used as (C, N): already the transpose of xt.
  - qT, kT are produced in (inner, N) layout; V in (N, inner) layout with an
    extra ones-column per head so the P@V matmul also emits the softmax
    denominators (M = 64+1 = 65).
  - simT tiles are (key j on partitions, query i on free); exp runs on
    ScalarE straight out of PSUM into bf16 SBUF. Attention steady state is
    paced by the 8 exp ops per head (~1.3us each).
  - Softmax denominators: bounce through DRAM in contiguous 8-elem chunks to
    spread the row across 128 partitions, reciprocal there, then a 0-stride
    DMA replicates 1/s for the normalize multiply. The last attn@v matmul +
    PSUM readout of head h fire early in head h+1 and the recip+mul a head
    after that, so chain latency never blocks the in-order PE/DVE queues.
  - PSUM budget (8 banks): 4 for sim/projection tiles (shared tag, double
    buffered) + 4 for two in-flight attn@v accumulators.
"""

import numpy as np

import concourse.bass as bass
import concourse.mybir as mybir
import concourse.tile as tile
from concourse import bacc

B, C, N = 8, 512, 1024
HEADS, D = 8, 64
INNER = HEADS * D  # 512
SCALE = D ** -0.5
P = 128
CT = C // P       # 4  k-tiles over C
MT = INNER // P   # 4  partition-tiles over inner
JT = N // P       # 8  key tiles
NT = N // P       # 8  output row tiles
NB = N // 512     # 2  free-dim blocks of 512 over N

F32 = mybir.dt.float32
BF16 = mybir.dt.bfloat16
EXP = mybir.ActivationFunctionType.Exp


def build_nc(debug=False):
    nc = bacc.Bacc(
        "TRN2", target_bir_lowering=False, debug=debug, num_devices=B
    )
    x_d = nc.dram_tensor("x", [C, N], F32, kind="ExternalInput")
    wq_d = nc.dram_tensor("Wq", [C, INNER], F32, kind="ExternalInput")
    wkv_d = nc.dram_tensor("Wkv", [C, 2 * INNER], F32, kind="ExternalInput")
    wo_d = nc.dram_tensor("Wo", [INNER, C], F32, kind="ExternalInput")
    bo_d = nc.dram_tensor("bo", [C], F32, kind="ExternalInput")
    out_d = nc.dram_tensor("out", [N, C], F32, kind="ExternalOutput")

    with tile.TileContext(nc) as tc:
        with (
            tc.tile_pool(name="persist", bufs=1) as persist,
            tc.tile_pool(name="stage", bufs=1) as stage,
            tc.tile_pool(name="etp", bufs=3) as etp,
            tc.tile_pool(name="ovp", bufs=4) as ovp,
            tc.tile_pool(name="small", bufs=2) as small,
            tc.tile_pool(name="dramp", bufs=2, space="DRAM") as dramp,
            tc.tile_pool(name="psS", bufs=2, space="PSUM") as psS,
            tc.tile_pool(name="psO", bufs=2, space="PSUM") as psO,
        ):
            # ---------------- load + cast inputs ----------------
            # x / Wq / Wkv are loaded+cast per 128-row chunk so the first
            # projection matmuls start as soon as their chunks land.
            x_f = stage.tile([P, CT, N], F32, tag="st_x")
            x_b = persist.tile([P, CT, N], BF16)
            x_dv = x_d[:].rearrange("(a p) n -> p a n", p=P)
            wq_f = stage.tile([P, CT, INNER], F32, tag="st_q")
            wq_b = persist.tile([P, CT, INNER], BF16)
            wq_dv = wq_d[:].rearrange("(a p) m -> p a m", p=P)
            wkv_f = stage.tile([P, CT, 2 * INNER], F32, tag="st_kv")
            wkv_b = persist.tile([P, CT, 2 * INNER], BF16)
            wkv_dv = wkv_d[:].rearrange("(a p) m -> p a m", p=P)
            for a in range(CT):
                nc.sync.dma_start(out=x_f[:, a, :], in_=x_dv[:, a, :])
                nc.scalar.copy(out=x_b[:, a, :], in_=x_f[:, a, :])
                nc.sync.dma_start(out=wq_f[:, a, :], in_=wq_dv[:, a, :])
                nc.scalar.copy(out=wq_b[:, a, :], in_=wq_f[:, a, :])
            # Wkv is first needed by k-proj, which runs after all of q-proj;
            # loading it after x/Wq tightens the startup ramp
            for a in range(CT):
                nc.sync.dma_start(out=wkv_f[:, a, :], in_=wkv_dv[:, a, :])
                nc.scalar.copy(out=wkv_b[:, a, :], in_=wkv_f[:, a, :])

            wo_f = stage.tile([P, MT, C], F32, tag="st_q")
            nc.sync.dma_start(out=wo_f, in_=wo_d[:].rearrange("(a p) m -> p a m", p=P))
            wo_b = persist.tile([P, MT, C], BF16)
            nc.vector.tensor_copy(out=wo_b, in_=wo_f)

            bo_bc = persist.tile([P, C], F32)
            bo_ap = bo_d[:]
            nc.gpsimd.dma_start(
                out=bo_bc,
                in_=bass.AP(tensor=bo_ap.tensor, offset=bo_ap.offset,
                            ap=[[0, P], [1, C]]),
            )

            zb = persist.tile([P, 1], F32)
            nc.vector.memset(zb, 0.0)

            # ---------------- projections ----------------
            # qT, kT: (inner, N) transposed layout; inner = mt*128 + p.
            # Order: k/q for mt=0 first (gates head 0), then V (gates the
            # first attn@v), then the remaining k/q tiles.
            qT = persist.tile([P, MT, N], BF16)
            kT = persist.tile([P, MT, N], BF16)
            v_ext = persist.tile([P, JT, HEADS, D + 1], BF16)
            nc.vector.memset(v_ext[:, :, :, D], 1.0)

            def kq_proj(mt):
                for dst, w_b in ((kT, wkv_b), (qT, wq_b)):
                    for ib in range(NB):
                        psf = psS.tile([P, N], F32, tag="st")
                        ps = psf[:, 0:512]
                        for a in range(CT):
                            nc.tensor.matmul(
                                ps,
                                lhsT=w_b[:, a, mt * P:(mt + 1) * P],
                                rhs=x_b[:, a, ib * 512:(ib + 1) * 512],
                                start=(a == 0),
                                stop=(a == CT - 1),
                            )
                        nc.vector.tensor_copy(
                            out=dst[:, mt, ib * 512:(ib + 1) * 512], in_=ps)

            def v_proj():
                # V in normal layout (token j on partitions), per head with an
                # extra ones column: v_ext[:, jt, h, 0:64] = V, [..., 64] = 1
                for jt in range(JT):
                    psf = psS.tile([P, N], F32, tag="st")
                    ps = psf[:, 0:512]
                    for a in range(CT):
                        nc.tensor.matmul(
                            ps,
                            lhsT=x_b[:, a, jt * P:(jt + 1) * P],
                            rhs=wkv_b[:, a, INNER:2 * INNER],
                            start=(a == 0),
                            stop=(a == CT - 1),
                        )
                    nc.vector.tensor_copy(
                        out=v_ext[:, jt, :, 0:D],
                        in_=ps.rearrange("p (h d) -> p h d", h=HEADS),
                    )

            # dense projection phase first measures fastest overall: the PE
            # runs it uninterrupted while the attention pipeline spins up
            for mt in range(MT):
                kq_proj(mt)
            v_proj()

            # ---------------- attention (per head) ----------------
            # normalized O^T as one tile PER head-pair: gives the output
            # projection per-pair dependencies, so its kk<3 matmuls can
            # run while the last head's denominator chain drains
            oTs = []
            for m in range(MT):
                oT_m = persist.tile([P, N], BF16, tag=f"oT{m}")
                oTs.append(oT_m)
            pending_avtail = None   # emits av-tail of h-1, returns finish
            pending_finish = None   # finish of h-2
            for h in range(HEADS):
                hp = (h % 2) * D
                hm = h // 2
                qh = qT[hp:hp + D, hm, :]   # [64, N]
                kh = kT[hp:hp + D, hm, :]   # [64, N]

                et = etp.tile([P, JT, N], BF16, tag="et")
                for jt in range(JT):
                    st = psS.tile([P, N], F32, tag="st")
                    for ib in range(NB):
                        nc.tensor.matmul(
                            st[:, ib * 512:(ib + 1) * 512],
                            lhsT=kh[:, jt * P:(jt + 1) * P],
                            rhs=qh[:, ib * 512:(ib + 1) * 512],
                            start=True,
                            stop=True,
                        )
                    # E^T = exp(scale * S^T), PSUM -> bf16 SBUF
                    nc.scalar.activation(
                        out=et[:, jt, :], in_=st, func=EXP, bias=zb, scale=SCALE)
                    if jt == 1:
                        # previous head's attn@v tail + epilogue fire here so
                        # this head's first sim groups keep the exp stream
                        # bubble-free across the head boundary; the finish
                        # (recip+mul) of the head before that fires too
                        if pending_finish is not None:
                            pending_finish()
                            pending_finish = None
                        if pending_avtail is not None:
                            pending_finish = pending_avtail()
                            pending_avtail = None

                # O'^T_ext = [V_h | 1]^T @ E^T ; row D is the softmax denom
                # (last key tile + readout deferred into the next head)
                ov = psO.tile([D + 1, N], F32, tag="ov")
                for jt in range(JT - 1):
                    for ib in range(NB):
                        nc.tensor.matmul(
                            ov[:, ib * 512:(ib + 1) * 512],
                            lhsT=v_ext[:, jt, h, :],
                            rhs=et[:, jt, ib * 512:(ib + 1) * 512],
                            start=(jt == 0),
                            stop=False,
                        )

                def avtail(h=h, ov=ov, et=et):
                    jt = JT - 1
                    for ib in range(NB):
                        nc.tensor.matmul(
                            ov[:, ib * 512:(ib + 1) * 512],
                            lhsT=v_ext[:, jt, h, :],
                            rhs=et[:, jt, ib * 512:(ib + 1) * 512],
                            start=False,
                            stop=True,
                        )
                    # two quick copies release the PSUM tile; the s-row copy
                    # goes first since it gates the recip chain
                    s_tmp = small.tile([1, N], F32, tag="stmp")
                    nc.vector.tensor_copy(out=s_tmp, in_=ov[D:D + 1, :])
                    ov_sb = ovp.tile([D, N], BF16, tag="ovsb")
                    nc.vector.tensor_copy(out=ov_sb, in_=ov[0:D, :])
                    # 1/denom 128 lanes wide: bounce through DRAM to spread
                    # the row across partitions (contiguous 8-elem chunks)
                    sd = dramp.tile([N], F32, tag="sd")
                    nc.sync.dma_start(out=sd, in_=s_tmp)
                    st2 = small.tile([P, NT], F32, tag="st2")
                    nc.sync.dma_start(
                        out=st2, in_=sd.rearrange("(p k) -> p k", k=NT))

                    def finish(h=h, ov_sb=ov_sb, st2=st2):
                        rst2 = small.tile([P, NT], F32, tag="rst2")
                        nc.vector.reciprocal(rst2, st2)
                        rsd = dramp.tile([N], F32, tag="rsd")
                        nc.sync.dma_start(
                            out=rsd.rearrange("(p k) -> p k", k=NT), in_=rst2)
                        rep = small.tile([D, N], F32, tag="rep")
                        rsd_ap = rsd[:]
                        nc.sync.dma_start(
                            out=rep,
                            in_=bass.AP(tensor=rsd_ap.tensor,
                                        offset=rsd_ap.offset,
                                        ap=[[0, D], [1, N]]),
                        )
                        hp2 = (h % 2) * D
                        nc.vector.tensor_mul(
                            oTs[h // 2][hp2:hp2 + D, :], ov_sb, rep)

                    return finish

                pending_avtail = avtail

            # drain the deferral chain: finish(6), avtail(7), finish(7)
            if pending_finish is not None:
                pending_finish()
            pending_finish = pending_avtail()
            pending_finish()

            # ---------------- output projection ----------------
            for it in range(NT):
                pff = psS.tile([P, N], F32, tag="st")
                pf = pff[:, 0:C]
                for kk in range(MT):
                    nc.tensor.matmul(
                        pf,
                        lhsT=oTs[kk][:, it * P:(it + 1) * P],
                        rhs=wo_b[:, kk, :],
                        start=(kk == 0),
                        stop=(kk == MT - 1),
                    )
                fin = small.tile([P, C], F32, tag="fin")
                nc.vector.tensor_add(fin, pf, bo_bc)
                nc.sync.dma_start(out=out_d[it * P:(it + 1) * P, :], in_=fin)

    return nc


def kernel(x, Wq, Wkv, Wo, bo):
    from concourse.bass_utils import run_bass_kernel_spmd

    nc = build_nc()
    nc.compile()
    x = np.asarray(x)
    xs = np.ascontiguousarray(x.reshape(B, C, N)).astype(np.float32, copy=False)
    in_maps = [
        {
            "x": xs[b],
            "Wq": np.asarray(Wq, dtype=np.float32),
            "Wkv": np.asarray(Wkv, dtype=np.float32),
            "Wo": np.asarray(Wo, dtype=np.float32),
            "bo": np.asarray(bo, dtype=np.float32),
        }
        for b in range(B)
    ]
    res = run_bass_kernel_spmd(nc, in_maps, list(range(B)))
    return np.stack([res.results[b]["out"] for b in range(B)], axis=0)


# revision 89
# speedup vs baseline: 1.0095x; 1.0095x over previous
"""Self-contained Trainium2 Bass kernel for nn_Attention (8-head self-attention).

Reference computation (per batch element b):
    xt = x[b].reshape(C, N).T            # (N, C),  N = H*W = 1024
    q  = xt @ Wq                         # (N, 512)
    k, v = split(xt @ Wkv)               # (N, 512) each
    per head h (d=64): sim = q_h k_h^T / 8 ; P = softmax(sim) ; o_h = P v_h
    out[b] = concat_h(o_h) @ Wo + bo     # (N, C)

Sharding: pure data parallel -- core b computes batch element b (8 cores, 8
batch elements, no collectives).

Layout strategy (keeps every matmul contraction dim on partitions, zero
on-chip transposes):
  - x[b] is used as (C, N): already the transpose of xt.
  - qT, kT are produced in (inner, N) layout; V in (N, inner) layout with an
    extra ones-column per head so the P@V matmul also emits the softmax
    denominators (M = 64+1 = 65).
  - simT tiles are (key j on partitions, query i on free); exp runs on
    ScalarE straight out of PSUM into bf16 SBUF. Attention steady state is
    paced by the 8 exp ops per head (~1.3us each).
  - Softmax denominators: bounce through DRAM in contiguous 8-elem chunks to
    spread the row across 128 partitions, reciprocal there, then a 0-stride
    DMA replicates 1/s for the normalize multiply. The last attn@v matmul +
    PSUM readout of head h fire early in head h+1 and the recip+mul a head
    after that, so chain latency never blocks the in-order PE/DVE queues.
  - PSUM budget (8 banks): 4 for sim/projection tiles (shared tag, double
    buffered) + 4 for two in-flight attn@v accumulators.
"""

import numpy as np

import concourse.bass as bass
import concourse.mybir as mybir
import concourse.tile as tile
from concourse import bacc

B, C, N = 8, 512, 1024
HEADS, D = 8, 64
INNER = HEADS * D  # 512
SCALE = D ** -0.5
P = 128
CT = C // P       # 4  k-tiles over C
MT = INNER // P   # 4  partition-tiles over inner
JT = N // P       # 8  key tiles
NT = N // P       # 8  output row tiles
NB = N // 512     # 2  free-dim blocks of 512 over N

F32 = mybir.dt.float32
BF16 = mybir.dt.bfloat16
EXP = mybir.ActivationFunctionType.Exp


def build_nc(debug=False):
    nc = bacc.Bacc(
        "TRN2", target_bir_lowering=False, debug=debug, num_devices=B
    )
    x_d = nc.dram_tensor("x", [C, N], F32, kind="ExternalInput")
    wq_d = nc.dram_tensor("Wq", [C, INNER], F32, kind="ExternalInput")
    wkv_d = nc.dram_tensor("Wkv", [C, 2 * INNER], F32, kind="ExternalInput")
    wo_d = nc.dram_tensor("Wo", [INNER, C], F32, kind="ExternalInput")
    bo_d = nc.dram_tensor("bo", [C], F32, kind="ExternalInput")
    out_d = nc.dram_tensor("out", [N, C], F32, kind="ExternalOutput")

    with tile.TileContext(nc) as tc:
        with (
            tc.tile_pool(name="persist", bufs=1) as persist,
            tc.tile_pool(name="stage", bufs=1) as stage,
            tc.tile_pool(name="etp", bufs=3) as etp,
            tc.tile_pool(name="ovp", bufs=4) as ovp,
            tc.tile_pool(name="small", bufs=2) as small,
            tc.tile_pool(name="dramp", bufs=2, space="DRAM") as dramp,
            tc.tile_pool(name="psS", bufs=2, space="PSUM") as psS,
            tc.tile_pool(name="psO", bufs=2, space="PSUM") as psO,
        ):
            # ---------------- load + cast inputs ----------------
            # x / Wq / Wkv are loaded+cast per 128-row chunk so the first
            # projection matmuls start as soon as their chunks land.
            x_f = stage.tile([P, CT, N], F32, tag="st_x")
            x_b = persist.tile([P, CT, N], BF16)
            x_dv = x_d[:].rearrange("(a p) n -> p a n", p=P)
            wq_f = stage.tile([P, CT, INNER], F32, tag="st_q")
            wq_b = persist.tile([P, CT, INNER], BF16)
            wq_dv = wq_d[:].rearrange("(a p) m -> p a m", p=P)
            wkv_f = stage.tile([P, CT, 2 * INNER], F32, tag="st_kv")
            wkv_b = persist.tile([P, CT, 2 * INNER], BF16)
            wkv_dv = wkv_d[:].rearrange("(a p) m -> p a m", p=P)
            for a in range(CT):
                nc.sync.dma_start(out=x_f[:, a, :], in_=x_dv[:, a, :])
                nc.vector.tensor_copy(out=x_b[:, a, :], in_=x_f[:, a, :])
                nc.sync.dma_start(out=wq_f[:, a, :], in_=wq_dv[:, a, :])
                nc.scalar.copy(out=wq_b[:, a, :], in_=wq_f[:, a, :])
            # Wkv is first needed by k-proj, which runs after all of q-proj;
            # loading it after x/Wq tightens the startup ramp
            for a in range(CT):
                nc.sync.dma_start(out=wkv_f[:, a, :], in_=wkv_dv[:, a, :])
                nc.scalar.copy(out=wkv_b[:, a, :], in_=wkv_f[:, a, :])

            wo_f = stage.tile([P, MT, C], F32, tag="st_q")
            nc.sync.dma_start(out=wo_f, in_=wo_d[:].rearrange("(a p) m -> p a m", p=P))
            wo_b = persist.tile([P, MT, C], BF16)
            nc.vector.tensor_copy(out=wo_b, in_=wo_f)

            bo_bc = persist.tile([P, C], F32)
            bo_ap = bo_d[:]
            nc.gpsimd.dma_start(
                out=bo_bc,
                in_=bass.AP(tensor=bo_ap.tensor, offset=bo_ap.offset,
                            ap=[[0, P], [1, C]]),
            )

            zb = persist.tile([P, 1], F32)
            nc.vector.memset(zb, 0.0)

            # ---------------- projections ----------------
            # qT, kT: (inner, N) transposed layout; inner = mt*128 + p.
            # Order: k/q for mt=0 first (gates head 0), then V (gates the
            # first attn@v), then the remaining k/q tiles.
            qT = persist.tile([P, MT, N], BF16)
            kT = persist.tile([P, MT, N], BF16)
            v_ext = persist.tile([P, JT, HEADS, D + 1], BF16)
            nc.vector.memset(v_ext[:, :, :, D], 1.0)

            def kq_proj(mt):
                for dst, w_b in ((kT, wkv_b), (qT, wq_b)):
                    for ib in range(NB):
                        psf = psS.tile([P, N], F32, tag="st")
                        ps = psf[:, 0:512]
                        for a in range(CT):
                            nc.tensor.matmul(
                                ps,
                                lhsT=w_b[:, a, mt * P:(mt + 1) * P],
                                rhs=x_b[:, a, ib * 512:(ib + 1) * 512],
                                start=(a == 0),
                                stop=(a == CT - 1),
                            )
                        nc.vector.tensor_copy(
                            out=dst[:, mt, ib * 512:(ib + 1) * 512], in_=ps)

            def v_proj():
                # V in normal layout (token j on partitions), per head with an
                # extra ones column: v_ext[:, jt, h, 0:64] = V, [..., 64] = 1
                for jt in range(JT):
                    psf = psS.tile([P, N], F32, tag="st")
                    ps = psf[:, 0:512]
                    for a in range(CT):
                        nc.tensor.matmul(
                            ps,
                            lhsT=x_b[:, a, jt * P:(jt + 1) * P],
                            rhs=wkv_b[:, a, INNER:2 * INNER],
                            start=(a == 0),
                            stop=(a == CT - 1),
                        )
                    nc.vector.tensor_copy(
                        out=v_ext[:, jt, :, 0:D],
                        in_=ps.rearrange("p (h d) -> p h d", h=HEADS),
                    )

            # dense projection phase first measures fastest overall: the PE
            # runs it uninterrupted while the attention pipeline spins up
            for mt in range(MT):
                kq_proj(mt)
            v_proj()

            # ---------------- attention (per head) ----------------
            # normalized O^T as one tile PER head-pair: gives the output
            # projection per-pair dependencies, so its kk<3 matmuls can
            # run while the last head's denominator chain drains
            oTs = []
            for m in range(MT):
                oT_m = persist.tile([P, N], BF16, tag=f"oT{m}")
                oTs.append(oT_m)
            pending_avtail = None   # emits av-tail of h-1, returns finish
            pending_finish = None   # finish of h-2
            for h in range(HEADS):
                hp = (h % 2) * D
                hm = h // 2
                qh = qT[hp:hp + D, hm, :]   # [64, N]
                kh = kT[hp:hp + D, hm, :]   # [64, N]

                et = etp.tile([P, JT, N], BF16, tag="et")
                for jt in range(JT):
                    st = psS.tile([P, N], F32, tag="st")
                    for ib in range(NB):
                        nc.tensor.matmul(
                            st[:, ib * 512:(ib + 1) * 512],
                            lhsT=kh[:, jt * P:(jt + 1) * P],
                            rhs=qh[:, ib * 512:(ib + 1) * 512],
                            start=True,
                            stop=True,
                        )
                    # E^T = exp(scale * S^T), PSUM -> bf16 SBUF
                    nc.scalar.activation(
                        out=et[:, jt, :], in_=st, func=EXP, bias=zb, scale=SCALE)
                    if jt == 1:
                        # previous head's attn@v tail + epilogue fire here so
                        # this head's first sim groups keep the exp stream
                        # bubble-free across the head boundary; the finish
                        # (recip+mul) of the head before that fires too
                        if pending_finish is not None:
                            pending_finish()
                            pending_finish = None
                        if pending_avtail is not None:
                            pending_finish = pending_avtail()
                            pending_avtail = None

                # O'^T_ext = [V_h | 1]^T @ E^T ; row D is the softmax denom
                # (last key tile + readout deferred into the next head)
                ov = psO.tile([D + 1, N], F32, tag="ov")
                for jt in range(JT - 1):
                    for ib in range(NB):
                        nc.tensor.matmul(
                            ov[:, ib * 512:(ib + 1) * 512],
                            lhsT=v_ext[:, jt, h, :],
                            rhs=et[:, jt, ib * 512:(ib + 1) * 512],
                            start=(jt == 0),
                            stop=False,
                        )

                def avtail(h=h, ov=ov, et=et):
                    jt = JT - 1
                    for ib in range(NB):
                        nc.tensor.matmul(
                            ov[:, ib * 512:(ib + 1) * 512],
                            lhsT=v_ext[:, jt, h, :],
                            rhs=et[:, jt, ib * 512:(ib + 1) * 512],
                            start=False,
                            stop=True,
                        )
                    # two quick copies release the PSUM tile; the s-row copy
                    # goes first since it gates the recip chain
                    s_tmp = small.tile([1, N], F32, tag="stmp")
                    nc.vector.tensor_copy(out=s_tmp, in_=ov[D:D + 1, :])
                    ov_sb = ovp.tile([D, N], BF16, tag="ovsb")
                    nc.vector.tensor_copy(out=ov_sb, in_=ov[0:D, :])
                    # 1/denom 128 lanes wide: bounce through DRAM to spread
                    # the row across partitions (contiguous 8-elem chunks)
                    sd = dramp.tile([N], F32, tag="sd")
                    nc.sync.dma_start(out=sd, in_=s_tmp)
                    st2 = small.tile([P, NT], F32, tag="st2")
                    nc.sync.dma_start(
                        out=st2, in_=sd.rearrange("(p k) -> p k", k=NT))

                    def finish(h=h, ov_sb=ov_sb, st2=st2):
                        rst2 = small.tile([P, NT], F32, tag="rst2")
                        nc.vector.reciprocal(rst2, st2)
                        rsd = dramp.tile([N], F32, tag="rsd")
                        nc.sync.dma_start(
                            out=rsd.rearrange("(p k) -> p k", k=NT), in_=rst2)
                        rep = small.tile([D, N], F32, tag="rep")
                        rsd_ap = rsd[:]
                        nc.sync.dma_start(
                            out=rep,
                            in_=bass.AP(tensor=rsd_ap.tensor,
                                        offset=rsd_ap.offset,
                                        ap=[[0, D], [1, N]]),
                        )
                        hp2 = (h % 2) * D
                        nc.vector.tensor_mul(
                            oTs[h // 2][hp2:hp2 + D, :], ov_sb, rep)

                    return finish

                pending_avtail = avtail

            # drain the deferral chain: finish(6), avtail(7), finish(7)
            if pending_finish is not None:
                pending_finish()
            pending_finish = pending_avtail()
            pending_finish()

            # ---------------- output projection ----------------
            for it in range(NT):
                pff = psS.tile([P, N], F32, tag="st")
                pf = pff[:, 0:C]
                for kk in range(MT):
                    nc.tensor.matmul(
                        pf,
                        lhsT=oTs[kk][:, it * P:(it + 1) * P],
                        rhs=wo_b[:, kk, :],
                        start=(kk == 0),
                        stop=(kk == MT - 1),
                    )
                fin = small.tile([P, C], F32, tag="fin")
                nc.vector.tensor_add(fin, pf, bo_bc)
                nc.sync.dma_start(out=out_d[it * P:(it + 1) * P, :], in_=fin)

    return nc


def kernel(x, Wq, Wkv, Wo, bo):
    from concourse.bass_utils import run_bass_kernel_spmd

    nc = build_nc()
    nc.compile()
    x = np.asarray(x)
    xs = np.ascontiguousarray(x.reshape(B, C, N)).astype(np.float32, copy=False)
    in_maps = [
        {
            "x": xs[b],
            "Wq": np.asarray(Wq, dtype=np.float32),
            "Wkv": np.asarray(Wkv, dtype=np.float32),
            "Wo": np.asarray(Wo, dtype=np.float32),
            "bo": np.asarray(bo, dtype=np.float32),
        }
        for b in range(B)
    ]
    res = run_bass_kernel_spmd(nc, in_maps, list(range(B)))
    return np.stack([res.results[b]["out"] for b in range(B)], axis=0)


# revision 90
# speedup vs baseline: 1.0265x; 1.0169x over previous
"""Self-contained Trainium2 Bass kernel for nn_Attention (8-head self-attention).

Reference computation (per batch element b):
    xt = x[b].reshape(C, N).T            # (N, C),  N = H*W = 1024
    q  = xt @ Wq                         # (N, 512)
    k, v = split(xt @ Wkv)               # (N, 512) each
    per head h (d=64): sim = q_h k_h^T / 8 ; P = softmax(sim) ; o_h = P v_h
    out[b] = concat_h(o_h) @ Wo + bo     # (N, C)

Sharding: pure data parallel -- core b computes batch element b (8 cores, 8
batch elements, no collectives).

Layout strategy (keeps every matmul contraction dim on partitions, zero
on-chip transposes):
  - x[b] is used as (C, N): already the transpose of xt.
  - qT, kT are produced in (inner, N) layout; V in (N, inner) layout with an
    extra ones-column per head so the P@V matmul also emits the softmax
    denominators (M = 64+1 = 65).
  - simT tiles are (key j on partitions, query i on free); exp runs on
    ScalarE straight out of PSUM into bf16 SBUF. Attention steady state is
    paced by the 8 exp ops per head (~1.3us each).
  - Softmax denominators: bounce through DRAM in contiguous 8-elem chunks to
    spread the row across 128 partitions, reciprocal there, then a 0-stride
    DMA replicates 1/s for the normalize multiply. The last attn@v matmul +
    PSUM readout of head h fire early in head h+1 and the recip+mul a head
    after that, so chain latency never blocks the in-order PE/DVE queues.
  - PSUM budget (8 banks): 4 for sim/projection tiles (shared tag, double
    buffered) + 4 for two in-flight attn@v accumulators.
"""

import numpy as np

import concourse.bass as bass
import concourse.mybir as mybir
import concourse.tile as tile
from concourse import bacc

B, C, N = 8, 512, 1024
HEADS, D = 8, 64
INNER = HEADS * D  # 512
SCALE = D ** -0.5
P = 128
CT = C // P       # 4  k-tiles over C
MT = INNER // P   # 4  partition-tiles over inner
JT = N // P       # 8  key tiles
NT = N // P       # 8  output row tiles
NB = N // 512     # 2  free-dim blocks of 512 over N

F32 = mybir.dt.float32
BF16 = mybir.dt.bfloat16
EXP = mybir.ActivationFunctionType.Exp


def build_nc(debug=False):
    nc = bacc.Bacc(
        "TRN2", target_bir_lowering=False, debug=debug, num_devices=B
    )
    x_d = nc.dram_tensor("x", [C, N], F32, kind="ExternalInput")
    wq_d = nc.dram_tensor("Wq", [C, INNER], F32, kind="ExternalInput")
    wkv_d = nc.dram_tensor("Wkv", [C, 2 * INNER], F32, kind="ExternalInput")
    wo_d = nc.dram_tensor("Wo", [INNER, C], F32, kind="ExternalInput")
    bo_d = nc.dram_tensor("bo", [C], F32, kind="ExternalInput")
    out_d = nc.dram_tensor("out", [N, C], F32, kind="ExternalOutput")

    with tile.TileContext(nc) as tc:
        with (
            tc.tile_pool(name="persist", bufs=1) as persist,
            tc.tile_pool(name="stage", bufs=1) as stage,
            tc.tile_pool(name="etp", bufs=3) as etp,
            tc.tile_pool(name="ovp", bufs=4) as ovp,
            tc.tile_pool(name="small", bufs=3) as small,
            tc.tile_pool(name="dramp", bufs=2, space="DRAM") as dramp,
            tc.tile_pool(name="psS", bufs=2, space="PSUM") as psS,
            tc.tile_pool(name="psO", bufs=2, space="PSUM") as psO,
        ):
            # ---------------- load + cast inputs ----------------
            # x / Wq / Wkv are loaded+cast per 128-row chunk so the first
            # projection matmuls start as soon as their chunks land.
            x_f = stage.tile([P, CT, N], F32, tag="st_x")
            x_b = persist.tile([P, CT, N], BF16)
            x_dv = x_d[:].rearrange("(a p) n -> p a n", p=P)
            wq_f = stage.tile([P, CT, INNER], F32, tag="st_q")
            wq_b = persist.tile([P, CT, INNER], BF16)
            wq_dv = wq_d[:].rearrange("(a p) m -> p a m", p=P)
            wkv_f = stage.tile([P, CT, 2 * INNER], F32, tag="st_kv")
            wkv_b = persist.tile([P, CT, 2 * INNER], BF16)
            wkv_dv = wkv_d[:].rearrange("(a p) m -> p a m", p=P)
            for a in range(CT):
                nc.sync.dma_start(out=x_f[:, a, :], in_=x_dv[:, a, :])
                nc.vector.tensor_copy(out=x_b[:, a, :], in_=x_f[:, a, :])
                nc.sync.dma_start(out=wq_f[:, a, :], in_=wq_dv[:, a, :])
                nc.scalar.copy(out=wq_b[:, a, :], in_=wq_f[:, a, :])
            # Wkv is first needed by k-proj, which runs after all of q-proj;
            # loading it after x/Wq tightens the startup ramp
            for a in range(CT):
                nc.sync.dma_start(out=wkv_f[:, a, :], in_=wkv_dv[:, a, :])
                nc.scalar.copy(out=wkv_b[:, a, :], in_=wkv_f[:, a, :])

            wo_f = stage.tile([P, MT, C], F32, tag="st_q")
            nc.sync.dma_start(out=wo_f, in_=wo_d[:].rearrange("(a p) m -> p a m", p=P))
            wo_b = persist.tile([P, MT, C], BF16)
            nc.vector.tensor_copy(out=wo_b, in_=wo_f)

            bo_bc = persist.tile([P, C], F32)
            bo_ap = bo_d[:]
            nc.gpsimd.dma_start(
                out=bo_bc,
                in_=bass.AP(tensor=bo_ap.tensor, offset=bo_ap.offset,
                            ap=[[0, P], [1, C]]),
            )

            zb = persist.tile([P, 1], F32)
            nc.vector.memset(zb, 0.0)

            # ---------------- projections ----------------
            # qT, kT: (inner, N) transposed layout; inner = mt*128 + p.
            # Order: k/q for mt=0 first (gates head 0), then V (gates the
            # first attn@v), then the remaining k/q tiles.
            qT = persist.tile([P, MT, N], BF16)
            kT = persist.tile([P, MT, N], BF16)
            v_ext = persist.tile([P, JT, HEADS, D + 1], BF16)
            nc.vector.memset(v_ext[:, :, :, D], 1.0)

            def kq_proj(mt):
                for dst, w_b in ((kT, wkv_b), (qT, wq_b)):
                    for ib in range(NB):
                        psf = psS.tile([P, N], F32, tag="st")
                        ps = psf[:, 0:512]
                        for a in range(CT):
                            nc.tensor.matmul(
                                ps,
                                lhsT=w_b[:, a, mt * P:(mt + 1) * P],
                                rhs=x_b[:, a, ib * 512:(ib + 1) * 512],
                                start=(a == 0),
                                stop=(a == CT - 1),
                            )
                        nc.vector.tensor_copy(
                            out=dst[:, mt, ib * 512:(ib + 1) * 512], in_=ps)

            def v_proj():
                # V in normal layout (token j on partitions), per head with an
                # extra ones column: v_ext[:, jt, h, 0:64] = V, [..., 64] = 1
                for jt in range(JT):
                    psf = psS.tile([P, N], F32, tag="st")
                    ps = psf[:, 0:512]
                    for a in range(CT):
                        nc.tensor.matmul(
                            ps,
                            lhsT=x_b[:, a, jt * P:(jt + 1) * P],
                            rhs=wkv_b[:, a, INNER:2 * INNER],
                            start=(a == 0),
                            stop=(a == CT - 1),
                        )
                    nc.vector.tensor_copy(
                        out=v_ext[:, jt, :, 0:D],
                        in_=ps.rearrange("p (h d) -> p h d", h=HEADS),
                    )

            # dense projection phase first measures fastest overall: the PE
            # runs it uninterrupted while the attention pipeline spins up
            for mt in range(MT):
                kq_proj(mt)
            v_proj()

            # ---------------- attention (per head) ----------------
            # normalized O^T as one tile PER head-pair: gives the output
            # projection per-pair dependencies, so its kk<3 matmuls can
            # run while the last head's denominator chain drains
            oTs = []
            for m in range(MT):
                oT_m = persist.tile([P, N], BF16, tag=f"oT{m}")
                oTs.append(oT_m)
            pending_avtail = None   # emits av-tail of h-1, returns finish
            pending_finish = None   # finish of h-2
            for h in range(HEADS):
                hp = (h % 2) * D
                hm = h // 2
                qh = qT[hp:hp + D, hm, :]   # [64, N]
                kh = kT[hp:hp + D, hm, :]   # [64, N]

                et = etp.tile([P, JT, N], BF16, tag="et")
                for jt in range(JT):
                    st = psS.tile([P, N], F32, tag="st")
                    for ib in range(NB):
                        nc.tensor.matmul(
                            st[:, ib * 512:(ib + 1) * 512],
                            lhsT=kh[:, jt * P:(jt + 1) * P],
                            rhs=qh[:, ib * 512:(ib + 1) * 512],
                            start=True,
                            stop=True,
                        )
                    # E^T = exp(scale * S^T), PSUM -> bf16 SBUF
                    nc.scalar.activation(
                        out=et[:, jt, :], in_=st, func=EXP, bias=zb, scale=SCALE)
                    if jt == 1:
                        # previous head's attn@v tail + epilogue fire here so
                        # this head's first sim groups keep the exp stream
                        # bubble-free across the head boundary; the finish
                        # (recip+mul) of the head before that fires too
                        if pending_finish is not None:
                            pending_finish()
                            pending_finish = None
                        if pending_avtail is not None:
                            pending_finish = pending_avtail()
                            pending_avtail = None

                # O'^T_ext = [V_h | 1]^T @ E^T ; row D is the softmax denom
                # (last key tile + readout deferred into the next head)
                ov = psO.tile([D + 1, N], F32, tag="ov")
                for jt in range(JT - 1):
                    for ib in range(NB):
                        nc.tensor.matmul(
                            ov[:, ib * 512:(ib + 1) * 512],
                            lhsT=v_ext[:, jt, h, :],
                            rhs=et[:, jt, ib * 512:(ib + 1) * 512],
                            start=(jt == 0),
                            stop=False,
                        )

                def avtail(h=h, ov=ov, et=et):
                    jt = JT - 1
                    for ib in range(NB):
                        nc.tensor.matmul(
                            ov[:, ib * 512:(ib + 1) * 512],
                            lhsT=v_ext[:, jt, h, :],
                            rhs=et[:, jt, ib * 512:(ib + 1) * 512],
                            start=False,
                            stop=True,
                        )
                    # two quick copies release the PSUM tile; the s-row copy
                    # goes first since it gates the recip chain
                    s_tmp = small.tile([1, N], F32, tag="stmp")
                    nc.vector.tensor_copy(out=s_tmp, in_=ov[D:D + 1, :])
                    ov_sb = ovp.tile([D, N], BF16, tag="ovsb")
                    nc.vector.tensor_copy(out=ov_sb, in_=ov[0:D, :])
                    # 1/denom 128 lanes wide: bounce through DRAM to spread
                    # the row across partitions (contiguous 8-elem chunks)
                    sd = dramp.tile([N], F32, tag="sd")
                    nc.sync.dma_start(out=sd, in_=s_tmp)
                    st2 = small.tile([P, NT], F32, tag="st2")
                    nc.sync.dma_start(
                        out=st2, in_=sd.rearrange("(p k) -> p k", k=NT))

                    def finish(h=h, ov_sb=ov_sb, st2=st2):
                        rst2 = small.tile([P, NT], F32, tag="rst2")
                        nc.vector.reciprocal(rst2, st2)
                        rsd = dramp.tile([N], F32, tag="rsd")
                        nc.sync.dma_start(
                            out=rsd.rearrange("(p k) -> p k", k=NT), in_=rst2)
                        rep = small.tile([D, N], F32, tag="rep")
                        rsd_ap = rsd[:]
                        nc.sync.dma_start(
                            out=rep,
                            in_=bass.AP(tensor=rsd_ap.tensor,
                                        offset=rsd_ap.offset,
                                        ap=[[0, D], [1, N]]),
                        )
                        hp2 = (h % 2) * D
                        nc.vector.tensor_mul(
                            oTs[h // 2][hp2:hp2 + D, :], ov_sb, rep)

                    return finish

                pending_avtail = avtail

            # drain the deferral chain: finish(6), avtail(7), finish(7)
            if pending_finish is not None:
                pending_finish()
            pending_finish = pending_avtail()
            pending_finish()

            # ---------------- output projection ----------------
            for it in range(NT):
                pff = psS.tile([P, N], F32, tag="st")
                pf = pff[:, 0:C]
                for kk in range(MT):
                    nc.tensor.matmul(
                        pf,
                        lhsT=oTs[kk][:, it * P:(it + 1) * P],
                        rhs=wo_b[:, kk, :],
                        start=(kk == 0),
                        stop=(kk == MT - 1),
                    )
                fin = small.tile([P, C], F32, tag="fin")
                nc.vector.tensor_add(fin, pf, bo_bc)
                nc.sync.dma_start(out=out_d[it * P:(it + 1) * P, :], in_=fin)

    return nc


def kernel(x, Wq, Wkv, Wo, bo):
    from concourse.bass_utils import run_bass_kernel_spmd

    nc = build_nc()
    nc.compile()
    x = np.asarray(x)
    xs = np.ascontiguousarray(x.reshape(B, C, N)).astype(np.float32, copy=False)
    in_maps = [
        {
            "x": xs[b],
            "Wq": np.asarray(Wq, dtype=np.float32),
            "Wkv": np.asarray(Wkv, dtype=np.float32),
            "Wo": np.asarray(Wo, dtype=np.float32),
            "bo": np.asarray(bo, dtype=np.float32),
        }
        for b in range(B)
    ]
    res = run_bass_kernel_spmd(nc, in_maps, list(range(B)))
    return np.stack([res.results[b]["out"] for b in range(B)], axis=0)
